# revision 2
# baseline (speedup 1.0000x reference)
"""Trainium2 Bass kernel for nn_Ensemble_FC (BatchEnsemble fully-connected layer).

Math (reference):
    emb   = relu(alpha @ enc1_w.T + enc1_b)          # (M, H)
    mu    = emb @ encm_w.T + encm_b                  # (M, H)
    z     = eps * exp(0.5 * mu) + mu
    adec  = z @ dec_w.T + dec_b                      # (M, IN)
    out[m*B+i, o] = (sum_k x[i,k] * adec[m,k] * fc_w[o,k]) * gamma[m,o] + bias_p[m,o]

The VAE encoder (~1M MACs, 0.003% of total FLOPs) runs on the HOST in f32;
the device kernel does only the main GEMM (2048 x 4096 x 4096).

Sharding: tensor-parallel column-split of fc_w / gamma / bias_p over
out_features (4096 -> 8 x 512).  Every core computes the full
(M*B = 2048)-row GEMM for its 512 output columns.

On-chip layout is transposed ([feature, row]) so per-model scales
(adec, gamma, bias) are per-partition scalars:
    out_core[o_local, m*B+i] = psum * gamma + bias,
    psum = sum_kc  wT[kc, o-chunk].T @ (xT[kc] * adecT[kc, m])
GEMM runs in bf16 (rounded on host), fp32 PSUM accumulation; epilogues fp32.

Perf structure (trace-driven):
- ~7us fixed prologue (runtime barriers + IRAM loads) before any
  instruction runs, ~6.5us Tile drain + EVSEM-butterfly tail.
- Tiny constants (adecT scales, gamma/bias) ride the ACT HWDGE ring;
  BOTH bulk streams (x on the SP HWDGE ring, w on the Pool SWDGE ring)
  carry an explicit dep on the const DMA so the constants transfer with
  full 16-SDMA-engine bandwidth before the 8MB bulk flood starts.
- PE warm-up matmuls bridge the first bulk-DMA group's latency and trip
  the HAM clock gate (cold PE runs at 1.2 GHz; >3.4us of sustained
  work => full rate).
- Main GEMM runs in two passes: pass A (output chunks 0-1, all models,
  k-outer) starts on the first small DMA groups; pass B (chunks 2-3,
  group-major over scaled activations shared by both chunks) runs once
  data is resident and staggers group completions so the final
  epilogue + store tail is tiny.
"""

import os
import sys

for _p in ("/opt/trn_rl_repo",):
    if os.path.isdir(_p) and _p not in sys.path:
        sys.path.insert(0, _p)

import numpy as np
import ml_dtypes

import concourse.bass as bass  # noqa: F401  (registers engine libraries)
import concourse.mybir as mybir
import concourse.tile as tile
from concourse import bacc
from concourse.bass_utils import run_bass_kernel_spmd

N_CORES = 8
M = 4          # ensemble members
B = 512        # batch
IN = 4096      # in_features (contraction)
OUT = 4096     # out_features
H = 32         # encoder hidden
P = 128        # partitions
KC = IN // P   # 32 contraction chunks of 128
O_CORE = OUT // N_CORES   # 512 output columns per core
OC = O_CORE // P          # 4 o-chunks of 128 per core
N_WARM = 8     # PE warm-up matmuls (~3.4us cold => HAM unthrottled)

# bulk-stream DMA groups (kc each); small head groups so the first
# matmuls aren't gated on a full 512KB transfer per stream
GROUP_KCS = [2, 2, 4, 4, 4, 4, 4, 4, 4]
G = len(GROUP_KCS)
GROUP_OF_K = []
for _g, _n in enumerate(GROUP_KCS):
    GROUP_OF_K += [(_g, _j) for _j in range(_n)]
GROUP_K0 = [sum(GROUP_KCS[:_g]) for _g in range(G)]

# gb32 column layout (f32, [128, GB_W])
GB_G = 0                      # [p, oc, m]  OC*M = 16
GB_B = GB_G + OC * M
GB_W = GB_B + OC * M          # 32

F32 = mybir.dt.float32
BF16 = mybir.dt.bfloat16
AF = mybir.ActivationFunctionType
ALU = mybir.AluOpType

_nc_cache = {}


def _build_nc():
    """Build and compile the per-core Bass/Tile program (SPMD, same on all 8)."""
    nc = bacc.Bacc("TRN2", num_devices=N_CORES, debug=False)

    xh_d = nc.declare_dram_parameter("xh", [P, KC, B], BF16, isOutput=False)
    wh_d = nc.declare_dram_parameter("wh", [P, KC, O_CORE], BF16, isOutput=False)
    ad32_d = nc.declare_dram_parameter("ad32", [P, KC * M], F32, isOutput=False)
    gb32_d = nc.declare_dram_parameter("gb32", [P, GB_W], F32, isOutput=False)
    out_d = nc.declare_dram_parameter("out", [O_CORE, M * B], F32, isOutput=True)

    with tile.TileContext(nc) as tc:
        with (
            tc.tile_pool(name="consts", bufs=1) as consts,
            tc.tile_pool(name="xt", bufs=G) as xt_pool,
            tc.tile_pool(name="wt", bufs=G) as wt_pool,
            tc.tile_pool(name="xa", bufs=8) as xa_pool,
            tc.tile_pool(name="xa3", bufs=KC) as xa3_pool,
            tc.tile_pool(name="ps", bufs=8, space="PSUM") as ps_pool,
            tc.tile_pool(name="osb", bufs=4) as out_pool,
        ):
            # ---- PE warm-up: garbage matmuls bridge the bulk-DMA latency
            # and trip the HAM activity monitor (1.2 GHz -> full rate).
            wu_src = consts.tile([P, B], BF16)
            nc.gpsimd.memset(wu_src[:], 0.0)

            wu_ps = ps_pool.tile([P, B], F32, tag="ps")
            for i in range(N_WARM):
                nc.tensor.matmul(
                    wu_ps[:], lhsT=wu_src[:, :P], rhs=wu_src[:], start=True, stop=True
                )

            # ---- DMA issue.  The tiny constants go first on the ACT ring;
            # the bulk x/w streams (SP ring / Pool SWDGE ring) are held
            # behind them so the scales aren't starved by the 8 MB bulk
            # flood (SDMA engines round-robin at packet granularity).
            ad32_sb = consts.tile([P, KC * M], F32)
            ad_dma = nc.scalar.dma_start(ad32_sb[:], ad32_d.ap())
            gb32_sb = consts.tile([P, GB_W], F32)
            nc.scalar.dma_start(gb32_sb[:], gb32_d.ap())
            xt_tiles = []
            wt_tiles = []
            for g in range(G):
                ks = slice(GROUP_K0[g], GROUP_K0[g] + GROUP_KCS[g])
                xt = xt_pool.tile([P, GROUP_KCS[g], B], BF16, tag="xt")
                xdma = nc.sync.dma_start(xt[:], xh_d.ap()[:, ks, :])
                wt = wt_pool.tile([P, GROUP_KCS[g], O_CORE], BF16, tag="wt")
                wdma = nc.gpsimd.dma_start(wt[:], wh_d.ap()[:, ks, :])
                if g == 0:
                    tile.add_dep_helper(
                        xdma.ins, ad_dma.ins, reason="x stream after consts"
                    )
                    tile.add_dep_helper(
                        wdma.ins, ad_dma.ins, reason="w stream after consts"
                    )
                xt_tiles.append(xt)
                wt_tiles.append(wt)

            g_v = gb32_sb[:, GB_G:GB_B].rearrange("p (o m) -> p o m", m=M)
            b_v = gb32_sb[:, GB_B:GB_W].rearrange("p (o m) -> p o m", m=M)

            # consume the warm-up psum so bacc DCE keeps the warm-up,
            # without blocking any queue at the head.
            wu_sink = consts.tile([P, B], F32)
            nc.vector.tensor_copy(wu_sink[:], wu_ps[:])

            def epilogue(ps, oc, m, name):
                osb = out_pool.tile([P, B], F32, tag="osb", name=name)
                nc.scalar.activation(
                    osb[:],
                    ps[:],
                    AF.Identity,
                    bias=b_v[:, oc, m : m + 1],
                    scale=g_v[:, oc, m : m + 1],
                )
                nc.sync.dma_start(
                    out_d.ap()[oc * P : (oc + 1) * P, m * B : (m + 1) * B],
                    osb[:],
                )

            # ---- main GEMM.
            # Pass A: oc in {0,1} x all models, k-outer — consumes each DMA
            # group as it lands, tracking the bulk-DMA arrival rate.
            A_OCS = (0, 1)
            psA = {
                (oc, m): ps_pool.tile([P, B], F32, tag="ps", name=f"psA_{oc}_{m}")
                for oc in A_OCS
                for m in range(M)
            }

            for k in range(KC):
                g, j = GROUP_OF_K[k]
                for m in range(M):
                    xa = xa_pool.tile([P, B], BF16, tag="xa", name=f"xaA_{k}_{m}")
                    nc.vector.tensor_scalar_mul(
                        xa[:], xt_tiles[g][:, j, :],
                        ad32_sb[:, k * M + m : k * M + m + 1],
                    )
                    for oc in A_OCS:
                        nc.tensor.matmul(
                            psA[(oc, m)][:],
                            lhsT=wt_tiles[g][:, j, oc * P : (oc + 1) * P],
                            rhs=xa[:],
                            start=(k == 0),
                            stop=(k == KC - 1),
                        )
            for m in range(M):
                for oc in A_OCS:
                    epilogue(psA[(oc, m)], oc, m, f"osbA_{oc}_{m}")

            # Pass B: oc in {2,3}, group-major (all data resident by now);
            # per model the 32 scaled tiles are materialized once and used
            # by both oc chunks, and group completions stagger so the final
            # epilogue + store tail is tiny.
            B_OCS = (2, 3)
            for m in range(M):
                xab_tiles = []
                for k in range(KC):
                    xab = xa3_pool.tile([P, B], BF16, tag="xa3", name=f"xaB_{m}_{k}")
                    nc.vector.tensor_scalar_mul(
                        xab[:], xt_tiles[GROUP_OF_K[k][0]][:, GROUP_OF_K[k][1], :],
                        ad32_sb[:, k * M + m : k * M + m + 1],
                    )
                    xab_tiles.append(xab)
                for oc in B_OCS:
                    psB = ps_pool.tile([P, B], F32, tag="ps", name=f"psB_{m}_{oc}")
                    for k in range(KC):
                        g, j = GROUP_OF_K[k]
                        nc.tensor.matmul(
                            psB[:],
                            lhsT=wt_tiles[g][:, j, oc * P : (oc + 1) * P],
                            rhs=xab_tiles[k][:],
                            start=(k == 0),
                            stop=(k == KC - 1),
                        )
                    epilogue(psB, oc, m, f"osbB_{m}_{oc}")

    nc.compile()
    return nc


def _get_nc():
    if "nc" not in _nc_cache:
        _nc_cache["nc"] = _build_nc()
    return _nc_cache["nc"]


def _pk(a2d):
    """(C*P, W) -> (P, C*W): row 128c+p -> [p, c, :] flattened."""
    c = a2d.shape[0] // P
    w = a2d.shape[1]
    return np.ascontiguousarray(
        a2d.reshape(c, P, w).transpose(1, 0, 2).reshape(P, c * w)
    )


def kernel(
    x, eps, alpha, gamma, bias_p, fc_w,
    enc1_w, enc1_b, encm_w, encm_b, dec_w, dec_b,
):
    bf16 = ml_dtypes.bfloat16
    f32 = np.float32
    asc = np.ascontiguousarray

    x = np.asarray(x, f32)
    fc_w = np.asarray(fc_w, f32)

    # ---- VAE encoder on host (f32): adec = dec(reparam(enc(alpha)))
    alpha_f = np.asarray(alpha, f32)
    emb = np.maximum(alpha_f @ np.asarray(enc1_w, f32).T + np.asarray(enc1_b, f32), 0.0)
    mu = emb @ np.asarray(encm_w, f32).T + np.asarray(encm_b, f32)
    z = np.asarray(eps, f32) * np.exp(0.5 * mu) + mu
    adec = (z @ np.asarray(dec_w, f32).T + np.asarray(dec_b, f32)).astype(f32)  # (M, IN)

    # x: (B, IN) -> xh (P, KC, B) bf16, xh[p,k,r] = x[r, 128k+p]
    xh = asc(x.astype(bf16).T.reshape(KC, P, B).transpose(1, 0, 2))
    # fc_w: (OUT, IN) -> per-core wh (P, KC, O_CORE) bf16
    wT_full = fc_w.astype(bf16).T  # (IN, OUT) view

    # adecT in per-partition-scalar layout: ad32[p, k*M+m] = adec[m, 128k+p]
    ad32 = _pk(asc(adec.T))  # (P, KC*M) f32

    gT_full = np.asarray(gamma, f32).T                    # (OUT, M)
    bT_full = np.asarray(bias_p, f32).T                   # (OUT, M)

    in_maps = []
    for c in range(N_CORES):
        o0, o1 = c * O_CORE, (c + 1) * O_CORE
        wh = asc(wT_full[:, o0:o1].reshape(KC, P, O_CORE).transpose(1, 0, 2))
        gb32 = np.empty((P, GB_W), f32)
        gb32[:, GB_G:GB_B] = _pk(asc(gT_full[o0:o1]))
        gb32[:, GB_B:GB_W] = _pk(asc(bT_full[o0:o1]))
        in_maps.append({"xh": xh, "wh": wh, "ad32": ad32, "gb32": gb32})

    nc = _get_nc()
    res = None
    for attempt in range(3):
        try:
            res = run_bass_kernel_spmd(nc, in_maps, list(range(N_CORES)))
            break
        except Exception:
            # transient NRT_EXEC_UNIT_UNRECOVERABLE wedges can follow an
            # earlier crashed process on the same cores; retry clears it
            if attempt == 2:
                raise
            import time

            time.sleep(5.0)
    outT = np.concatenate(
        [res.results[c]["out"] for c in range(N_CORES)], axis=0
    )  # (OUT, M*B)
    return asc(outT.T.astype(np.float32))  # (M*B, OUT)
